# revision 20
# baseline (speedup 1.0000x reference)
"""Trainium2 Bass kernel for nn_Attention (dense transformer MHA forward).

Shapes: x [4096, 1024], 16 heads x head_dim 64, full softmax attention.

Sharding (8 cores, tensor-parallel over heads): each core owns 2 heads.
  - column-parallel qkv: core computes q,k,v for its 2 heads only
  - local attention for 2 heads
  - row-parallel proj: core computes a partial [4096, 1024] output
  - "all-reduce" = host-side sum of the 8 partials (+ b_proj once)

Perf design:
  - all matmul operands bf16 (same PE rate as f32r, half the DMA/SBUF)
  - softmax exp split across engines: Activation handles 800 of each
    iteration's 1024 score columns, DVE the other 224 via two custom ops
    (Taylor-3 of e^{x/512}, then six squarings -> e^{x/8}, rel err ~2e-4).
    Each engine gets its own PSUM score tile (s_ps for ACT, sdve for DVE) so
    the Tile framework's same-tile reader serialization cannot couple them;
    S for head 1 is emitted as two matmuls to fill both tiles.
  - steady state is PE-bound at the row floor: 4.25 matmuls/iteration
    (2048 rows = 853 ns at 2.4 GHz), softmax engines ~95% subscribed
  - software pipeline with a 2-iteration PV lag; per-q-chunk tail
    (Z-recip on DVE, PE broadcast via sel-matmul, normalize into an SBUF
    OT buffer) bursts right after the last PV of the previous chunk
  - projection of OT deferred to an end-phase: PE matmuls with PSUM
    evictions alternating ACT/DVE and yp tiles alternating between the
    mpsum and (by then idle) spsum rings; y DMA'd per 128-row block
  - y partials written as bf16 (halved output DMA), summed on host in f32
"""

import numpy as np

SEQ = 4096
DIM = 1024
HEADS = 16
HD = 64
NCORES = 8
QCH = 512          # q-chunk (matmul moving free dim)
NQ = SEQ // QCH    # 8
NK = SEQ // 128    # 32 k-chunks
NDC = DIM // 128   # 8 contraction chunks for the qkv projection

DVE_COLS = 224            # exp columns handled by the DVE custom ops
K1 = 1.0 / 512.0          # exp via e^{x/512} ** 64 (scores carry a 1/8 scale)
K2 = K1 * K1 / 2.0
K3 = K1 * K1 * K1 / 6.0
LAG = 2                   # PV trails S by this many iterations
XS_SPLIT = True           # per-dc DMA granularity for the first s-chunk
YP_ALT = True             # alternate end-phase yp tiles between PSUM pools
SPS_BUFS = 2
E_BUFS = 4

_COMPILED = {}
_EXP_OPS = None


def _register_exp_ops():
    """Register two custom DVE ops computing exp(x/8) as SQ6(P3(x)):
    EXP_P3_ANT: p = ((C0 x + C1) x + C2) x + 1   (Taylor-3 of e^{x/512})
    SQ6_ANT:    y = p ** 64                       (six squarings)
    """
    global _EXP_OPS
    if _EXP_OPS is not None:
        return _EXP_OPS
    from concourse import dve_ops
    from concourse.dve_spec import C0, C1, C2, One, Spec, Src0, lower, sq
    from concourse.dve_uop import DveOpSpec

    def mk(name, body, ref):
        for op in dve_ops.OPS:
            if op.name == name:
                return op
        spec = Spec(body=body, reference=ref)
        if name not in dve_ops._SUB_OPCODE_FOR_NAME:
            row = dve_ops._CUSTOM_DVE_ROW_BASE + len(dve_ops._SUB_OPCODE_FOR_NAME)
            assert row < 0x20, "custom DVE opcode rows exhausted"
            dve_ops._SUB_OPCODE_FOR_NAME[name] = row
        shas = {}
        for ver in ("v3", "v4"):
            s = DveOpSpec(
                name=name,
                opcode=dve_ops.get_dve_sub_opcode(name),
                uops=lower(spec, ver=ver),
                rd1_en=False,
            )
            shas[ver] = s.sha(ver)
        op = dve_ops.DveOp(name, spec, subdim=False, uops_sha=shas)
        dve_ops.OPS.append(op)
        dve_ops.CUSTOM_DVE_SPECS[name] = spec
        return op

    p3 = mk(
        "EXP_P3_ANT",
        ((Src0 * C0 + C1) * Src0 + C2) * Src0 + One,
        lambda in0, in1, c0, c1, c2: ((in0 * c0 + c1) * in0 + c2) * in0 + 1.0,
    )
    sq6 = mk(
        "SQ6_ANT",
        sq(sq(sq(sq(sq(sq(Src0)))))),
        lambda in0, in1, c0, c1, c2: in0 ** 64,
    )
    _EXP_OPS = (p3, sq6)
    return _EXP_OPS


def _build_nc(loop_n=None, ablate=None):
    ACT_COLS = 2 * QCH - DVE_COLS
    import concourse.bacc as bacc
    from concourse import mybir, tile

    p3, sq6 = _register_exp_ops()

    f32 = mybir.dt.float32
    f32r = mybir.dt.float32r
    bf16 = mybir.dt.bfloat16
    nc = bacc.Bacc("TRN2", target_bir_lowering=False, debug=False)

    xT_d = nc.dram_tensor("xT", [NQ, 128, NDC, QCH], bf16, kind="ExternalInput")
    wqkvT_d = nc.dram_tensor("wqkvT", [128, NDC, 384], bf16, kind="ExternalInput")
    bq_d = nc.dram_tensor("bq", [128, 3], f32, kind="ExternalInput")
    wprojT_d = nc.dram_tensor("wprojT", [128, DIM], bf16, kind="ExternalInput")
    sel_d = nc.dram_tensor("sel", [128, 128], f32r, kind="ExternalInput")
    ident_d = nc.dram_tensor("ident", [128, 64], bf16, kind="ExternalInput")
    vfill_d = nc.dram_tensor("vfill", [128, NK, 65], bf16, kind="ExternalInput")
    zfill_d = nc.dram_tensor("zfill", [128, QCH], f32r, kind="ExternalInput")
    efill_d = None
    if ablate == "noexp":
        efill_d = nc.dram_tensor("efill", [128, 2 * QCH], bf16, kind="ExternalInput")
    y_d = nc.dram_tensor("y", [SEQ, DIM], bf16, kind="ExternalOutput")

    EXP = mybir.ActivationFunctionType.Exp
    COPY = mybir.ActivationFunctionType.Copy

    with tile.TileContext(nc) as tc, nc.allow_low_precision(
        reason="bf16 matmul inputs, fp32 PSUM accumulate; tol 2e-2"
    ):
        with (
            tc.tile_pool(name="const", bufs=1) as const,
            tc.tile_pool(name="xpool", bufs=3) as xpool,
            tc.tile_pool(name="big", bufs=1) as big,
            tc.tile_pool(name="epool", bufs=E_BUFS) as epool,
            tc.tile_pool(name="spool", bufs=3) as spool,
            tc.tile_pool(name="opool", bufs=2) as opool,
            tc.tile_pool(name="ypool", bufs=4) as ypool,
            tc.tile_pool(name="spsum", bufs=SPS_BUFS, space="PSUM") as spsum,
            tc.tile_pool(name="opsum", bufs=1, space="PSUM") as opsum,
            tc.tile_pool(name="mpsum", bufs=2, space="PSUM") as mpsum,
        ):
            # ---- constants ----
            wq = const.tile([128, NDC, 384], bf16)
            nc.sync.dma_start(out=wq[:, 0:2, :], in_=wqkvT_d.ap()[:, 0:2, :])
            nc.sync.dma_start(out=wq[:, 2:8, :], in_=wqkvT_d.ap()[:, 2:8, :])
            bq = const.tile([128, 3], f32)
            nc.sync.dma_start(out=bq, in_=bq_d.ap())
            sel = const.tile([128, 128], f32r)
            nc.sync.dma_start(out=sel, in_=sel_d.ap())
            idn = const.tile([128, 64], bf16)
            nc.sync.dma_start(out=idn, in_=ident_d.ap())
            wp = const.tile([128, DIM], bf16)
            nc.sync.dma_start(out=wp, in_=wprojT_d.ap())
            e_const = None
            if ablate == "noexp":
                e_const = const.tile([128, 2 * QCH], bf16, name="e_const")
                nc.sync.dma_start(out=e_const, in_=efill_d.ap())

            # ---- persistent SBUF state ----
            KT = big.tile([128, SEQ], bf16)   # rows 0:64 K^T h0, 64:128 K^T h1
            VT = big.tile([128, SEQ], bf16)
            QT = big.tile([128, SEQ], bf16)
            # per k-chunk stationary for P@V:
            #   cols 0:64 V_h0 | 64 ones | then h1 slab (65:193):
            #   local [0:32] zeros | [32] ones | [33:64] zeros | [64:128] V_h1
            vall = big.tile([128, NK, 193], bf16)
            zsb = big.tile([128, QCH], f32r)  # softmax-recip staging rows 32/64
            nc.sync.dma_start(out=zsb, in_=zfill_d.ap())
            nc.sync.dma_start(out=vall[:, :, 64:129], in_=vfill_d.ap())

            import contextlib
            loop_cm = (
                tc.For_i(0, loop_n, 1, hint_engines=(
                    mybir.EngineType.PE, mybir.EngineType.DVE,
                    mybir.EngineType.Activation, mybir.EngineType.SP,
                    mybir.EngineType.Pool,
                ))
                if loop_n else contextlib.nullcontext()
            )
            with loop_cm:
                xTr = xT_d.ap()
                dests = [KT, VT, QT]

                def emit_qkv_sc(sc):
                    xs = xpool.tile([128, NDC, QCH], bf16, tag="xs", name="xs")
                    if sc == 0 and XS_SPLIT:
                        for dc in range(NDC):
                            nc.sync.dma_start(
                                out=xs[:, dc:dc + 1, :], in_=xTr[sc, :, dc:dc + 1, :])
                    else:
                        nc.sync.dma_start(out=xs[:, 0:2, :], in_=xTr[sc, :, 0:2, :])
                        nc.sync.dma_start(out=xs[:, 2:5, :], in_=xTr[sc, :, 2:5, :])
                        nc.sync.dma_start(out=xs[:, 5:8, :], in_=xTr[sc, :, 5:8, :])
                    for m in range(3):
                        ps = mpsum.tile([128, QCH], f32, tag="mm", name="ps")
                        for dc in range(NDC):
                            nc.tensor.matmul(
                                ps,
                                lhsT=wq[:, dc, m * 128:(m + 1) * 128],
                                rhs=xs[:, dc, :],
                                start=(dc == 0),
                                stop=(dc == NDC - 1),
                            )
                        nc.vector.tensor_scalar_add(
                            dests[m][:, sc * QCH:(sc + 1) * QCH], ps, bq[:, m:m + 1]
                        )
                    # V^T -> V transposes for this s-chunk's 4 k-chunks
                    for kc in range(4 * sc, 4 * sc + 4):
                        for h in range(2):
                            tp = mpsum.tile([128, 64], bf16, tag="mm", name="tp")
                            nc.tensor.transpose(
                                tp, VT[64 * h:64 * h + 64, kc * 128:(kc + 1) * 128],
                                idn[64 * h:64 * h + 64, :]
                            )
                            dst = 0 if h == 0 else 129
                            nc.vector.tensor_copy(vall[:, kc, dst:dst + 64], tp)

                def emit_pv(e, kc, o0, o1):
                    nc.tensor.matmul(
                        o0[0:65, :], lhsT=vall[:, kc, 0:65], rhs=e[:, 0:QCH],
                        start=(kc == 0), stop=(kc == NK - 1),
                    )
                    nc.tensor.matmul(
                        o1, lhsT=vall[:, kc, 65:193], rhs=e[:, QCH:2 * QCH],
                        start=(kc == 0), stop=(kc == NK - 1),
                    )

                # normalized attention outputs, kept in SBUF until the
                # deferred projection end-phase
                OT = big.tile([128, SEQ], bf16)

                def make_tail(prev_qc, po0, po1):
                    """Tail of q-chunk prev_qc as a list of small emission
                    steps, consumed one per iteration of the next q-chunk.
                    Normalize straight out of PSUM on DVE; projection is
                    deferred to the end-phase."""
                    zbs = opool.tile([128, QCH], f32r, tag="zbs", name="zbs")
                    osl = slice(prev_qc * QCH, (prev_qc + 1) * QCH)
                    steps = [
                        lambda: (
                            nc.vector.reciprocal(zsb[64:65, :], po0[64:65, :]),
                            nc.vector.reciprocal(zsb[32:33, :], po1[32:33, :]),
                        ),
                    ]

                    def zb_step():
                        zb = mpsum.tile([128, QCH], f32, tag="mm", name="zb")
                        nc.tensor.matmul(zb, lhsT=sel, rhs=zsb, start=True, stop=True)
                        nc.vector.tensor_copy(zbs, zb)

                    steps.append(zb_step)
                    steps.append(lambda: nc.vector.tensor_mul(
                        OT[0:64, osl], po0[0:64, :], zbs[0:64, :]))
                    steps.append(lambda: nc.vector.tensor_mul(
                        OT[64:128, osl], po1[64:128, :], zbs[64:128, :]))
                    return steps

                def emit_proj(qc, ss, ev_dve):
                    # ev_dve: alternate yp between the mpsum ring and the
                    # (idle in the end-phase) spsum ring for a deeper pipeline
                    ysb = ypool.tile([128, DIM], bf16, tag="y", name="ysb")
                    for oh in range(2):
                        pool = spsum if (ev_dve and YP_ALT) else mpsum
                        tag = "s" if (ev_dve and YP_ALT) else "mm"
                        yp = pool.tile([128, QCH], f32, tag=tag, name="yp")
                        nc.tensor.matmul(
                            yp,
                            lhsT=OT[:, qc * QCH + ss * 128:qc * QCH + (ss + 1) * 128],
                            rhs=wp[:, oh * QCH:(oh + 1) * QCH],
                            start=True, stop=True,
                        )
                        dst = ysb[:, oh * QCH:(oh + 1) * QCH]
                        if oh == 1:
                            nc.vector.tensor_copy(dst, yp)
                        else:
                            nc.scalar.activation(dst, yp, COPY)
                    r0 = qc * QCH + ss * 128
                    nc.sync.dma_start(out=y_d.ap()[r0:r0 + 128, :], in_=ysb)

                pend_pv = []
                todo = []

                def attn_iter(qc, kc, o0, o1):
                    qsl = slice(qc * QCH, (qc + 1) * QCH)
                    ksl = slice(kc * 128, (kc + 1) * 128)
                    s_ps = spsum.tile([128, ACT_COLS], f32, tag="s", name="s_ps")
                    nc.tensor.matmul(
                        s_ps[:, 0:QCH], lhsT=KT[0:64, ksl], rhs=QT[0:64, qsl],
                        start=True, stop=True,
                    )
                    nc.tensor.matmul(
                        s_ps[:, QCH:ACT_COLS], lhsT=KT[64:128, ksl],
                        rhs=QT[64:128, qc * QCH:qc * QCH + ACT_COLS - QCH],
                        start=True, stop=True,
                    )
                    sdve = None
                    if DVE_COLS:
                        sdve = mpsum.tile([128, DVE_COLS], f32, tag="mm", name="sdve")
                        nc.tensor.matmul(
                            sdve, lhsT=KT[64:128, ksl],
                            rhs=QT[64:128, qc * QCH + ACT_COLS - QCH:(qc + 1) * QCH],
                            start=True, stop=True,
                        )
                    if ablate == "noexp":
                        e = e_const
                    else:
                        e = epool.tile([128, 2 * QCH], bf16, tag="e", name="e")
                        if DVE_COLS:
                            est = spool.tile(
                                [128, DVE_COLS], f32, tag="est", name="est")
                            nc.vector._custom_dve(
                                p3, out=est, in0=sdve,
                                s0=K3, s1=K2, imm2=K1)
                        nc.scalar.activation(
                            e[:, 0:ACT_COLS], s_ps, EXP, scale=0.125)
                        if DVE_COLS:
                            nc.vector._custom_dve(
                                sq6, out=e[:, ACT_COLS:2 * QCH], in0=est)
                    # the old generation's tail (which reads the o PSUM tiles)
                    # must be fully emitted before the first PV write of this
                    # generation (WAR on the single-buffered o tiles)
                    if kc == 2:
                        while todo:
                            todo.pop(0)()
                    pend_pv.append((e, kc, o0, o1))
                    if len(pend_pv) > LAG:
                        emit_pv(*pend_pv.pop(0))

                # qc 0 rides along with the qkv prologue: each s-chunk of qkv
                # unlocks 4 k-chunks of attention for q-chunk 0
                o0 = opsum.tile([128, QCH], f32, tag="o0", name="o0")
                o1 = opsum.tile([128, QCH], f32, tag="o1", name="o1")
                for sc in range(NQ):
                    emit_qkv_sc(sc)
                    for kc in range(4 * sc, 4 * sc + 4):
                        attn_iter(0, kc, o0, o1)

                for qc in range(1, NQ):
                    po0, po1 = o0, o1
                    o0 = opsum.tile([128, QCH], f32, tag="o0", name="o0")
                    o1 = opsum.tile([128, QCH], f32, tag="o1", name="o1")
                    todo.extend(make_tail(qc - 1, po0, po1))
                    for kc in range(NK):
                        attn_iter(qc, kc, o0, o1)

                while pend_pv:
                    emit_pv(*pend_pv.pop(0))
                while todo:
                    todo.pop(0)()
                for step in make_tail(NQ - 1, o0, o1):
                    step()

                # ---- deferred projection end-phase ----
                for qc in range(NQ):
                    for ss in range(4):
                        emit_proj(qc, ss, ev_dve=(ss % 2 == 1))

    nc.compile()
    return nc


def _bf16(a):
    import ml_dtypes
    return np.ascontiguousarray(a).astype(ml_dtypes.bfloat16)


def _prep_inputs(x, W_qkv, b_qkv, W_proj):
    """Host-side shard prep. Returns per-core input maps for the SPMD kernel."""
    # [sc, p, dc, q] layout: xt[sc, p, dc, q] = x[sc*512+q, dc*128+p]
    xT = _bf16(x.reshape(NQ, QCH, NDC, 128).transpose(0, 3, 2, 1))
    sel = np.zeros((128, 128), dtype=np.float32)
    sel[64, 0:64] = 1.0   # zsb partition 64 (recip Z0) -> bcast rows 0:64
    sel[32, 64:128] = 1.0  # zsb partition 32 (recip Z1) -> bcast rows 64:128
    ident = _bf16(np.vstack([np.eye(64, dtype=np.float32)] * 2))
    patt = np.zeros(65, dtype=np.float32)
    patt[0] = 1.0   # vall col 64: ones column for head 0 sums
    patt[33] = 1.0  # vall col 97: ones column for head 1 sums (partition 32)
    vfill = _bf16(np.broadcast_to(patt, (128, NK, 65)))
    zfill = np.zeros((128, QCH), dtype=np.float32)
    efill = np.ones((128, 2 * QCH), dtype=np.float32)

    in_maps = []
    for c in range(NCORES):
        h0 = 2 * c
        idx = np.concatenate([
            np.arange(DIM + HD * h0, DIM + HD * h0 + 128),          # K rows
            np.arange(2 * DIM + HD * h0, 2 * DIM + HD * h0 + 128),  # V rows
            np.arange(HD * h0, HD * h0 + 128),                      # Q rows
        ])
        w_shard = W_qkv[idx]                                  # [384, 1024]
        wqkvT = _bf16(w_shard.T.reshape(NDC, 128, 384).transpose(1, 0, 2))
        bqc = np.ascontiguousarray(b_qkv[idx].reshape(3, 128).T)  # [128, 3]
        wprojT = _bf16(W_proj[:, 128 * c:128 * (c + 1)].T)        # [128, 1024]
        in_maps.append({
            "xT": xT,
            "wqkvT": wqkvT,
            "bq": bqc,
            "wprojT": wprojT,
            "sel": sel,
            "ident": ident,
            "vfill": vfill,
            "zfill": zfill,
        })
        if ABLATE == "noexp":
            in_maps[-1]["efill"] = _bf16(efill)
    return in_maps


ABLATE = None


def _get_nc(loop_n=None):
    key = ("nc", loop_n, ABLATE)
    if key not in _COMPILED:
        _COMPILED[key] = _build_nc(loop_n, ablate=ABLATE)
    return _COMPILED[key]


def run(x, W_qkv, b_qkv, W_proj, b_proj, trace=False, **trace_kwargs):
    """Run the sharded kernel; returns (y_full, BassKernelResults)."""
    from concourse.bass_utils import run_bass_kernel_spmd

    x = np.asarray(x, dtype=np.float32)
    W_qkv = np.asarray(W_qkv, dtype=np.float32)
    b_qkv = np.asarray(b_qkv, dtype=np.float32)
    W_proj = np.asarray(W_proj, dtype=np.float32)
    b_proj = np.asarray(b_proj, dtype=np.float32)

    nc = _get_nc()
    in_maps = _prep_inputs(x, W_qkv, b_qkv, W_proj)
    res = run_bass_kernel_spmd(
        nc, in_maps, core_ids=list(range(NCORES)), trace=trace, **trace_kwargs
    )
    y = np.zeros((SEQ, DIM), dtype=np.float32)
    for r in res.results:
        y += np.asarray(r["y"], dtype=np.float32)
    y += b_proj
    return y, res


def kernel(x, W_qkv, b_qkv, W_proj, b_proj):
    y, _ = run(x, W_qkv, b_qkv, W_proj, b_proj, trace=False)
    return y
